# revision 28
# baseline (speedup 1.0000x reference)
"""Trainium2 Bass kernel for BatchedGNNModel (4-layer GCN over 3-rod chain graph).

Contract: kernel(**inputs) takes FULL unsharded inputs (as produced by
setup_inputs) and returns the FULL (64, 768, 3) float32 output.

Sharding: pure data parallel over batch — 8 items per NeuronCore on 8 cores,
identical SPMD program, weights/planes replicated (marshaled on host).

v3 fast-path algorithm (zero biases, expected adjacency structure):
  A_norm = D·M·D with D = diag(d), d = deg^-1/2, and M = tridiagonal-support
  ones + ~10 coefficient-1 sparse corrections (rigid-body couplings expressed
  through already-computed tri-window sums, rod-boundary removals). One
  application is S = tri_shift(U) + ents(U), U pre-scaled by d.

  All diagonal scales are algebraically folded so each A application costs
  exactly one plane multiply (or none):
    out = d ⊙ S4,  S4 = Tri(d² ⊙ S3),  S3 = Tri(d² ⊙ (H2 @ WC)),
    H2 = relu(S2),  S2 = Tri(H1' @ W2ᵀ),  H1' = relu((d² ⊙ Tri(d ⊙ x)) @ W1ᵀ)
  with WC = W3ᵀ W4ᵀ folded on host (feature transforms commute with node
  aggregation), d⊙x and the trailing d⊙ folded into host pre/post-processing.

  Engine split: PE does all feature matmuls in bf16 (weights stationary,
  activations feature-major, item-packed at partition stride 32, 4-item
  tile_position packing for the F=3 stage); DVE does the shifted adds /
  plane muls in bf16 (2x DVE mode), the sparse entries (5 two-column strided
  ops per pass), relu via tensor_scalar_max (4x mode), and a share of PSUM
  evacuation; ACT does most PSUM evacuation (relu/copy). GpSimd measured
  ~4ns/elem here and cannot access PSUM — unused. PSUM: one pool of
  [128,1024] f32 tiles x 4 bufs = 8 banks, deep enough that the PE runs
  ahead of the ACT/DVE evacuation chain. A PE warm-up burst covers the
  input-DMA + L1 window.

Fallback path (nonzero biases or unexpected adjacency): v1 dense program —
aggregations as PE matmuls against A_normᵀ / (A_norm@A_norm)ᵀ with bias
planes; slower but fully general.

This image's walrus accepts only one sync-wait slot per instruction, so a
post-pass splits Tile's multi-wait instructions into single-wait NoOps.
"""

import os
import sys

import numpy as np

sys.path.insert(0, "/opt/trn_rl_repo")

import ml_dtypes
import concourse.bass as bass
import concourse.mybir as mybir
import concourse.tile as _tile_mod
from concourse.tile import TileContext
from concourse.vector_clock import ScopedClock
from concourse.bass_utils import run_bass_kernel_spmd


def _patched_drain_and_barrier(self, tick_clock, wait_clock):
    """The nix walrus in this image only supports one sync-wait slot on a
    Drain; Tile's kernel-tail drain carries one wait per ticked semaphore.
    Split the extra waits onto single-wait nops on the same (sync) engine —
    program order makes this equivalent before the all-engine barrier."""
    drain_inst = self.nc.sync.drain()
    wait_clock.add_sem_waits(
        drain_inst.ins, ScopedClock({None: tick_clock.global_clock}))
    waits = list(drain_inst.ins.sync_info.on_wait)
    if len(waits) > 1:
        import bass_rust
        drain_inst.ins.sync_info.on_wait = [waits[0]]
        for w in waits[1:]:
            nop = self.nc.sync.nop(nofuse=True)
            si = nop.ins.sync_info
            if si is None:
                nop.ins.sync_info = bass_rust.SyncInfo(on_wait=[w], on_update=[])
            else:
                si.on_wait = [w]
    self.nc.all_engine_barrier()
    assert self.sems is not None
    popped = self.nc._tile_sem_poison_stack.pop()
    assert popped is self._sem_poison
    self.nc.clear_and_free_semaphores(list(self.sems.allocated().values()))
    self.nc.all_engine_barrier()


_tile_mod.TileContext._drain_and_barrier = _patched_drain_and_barrier


def _split_multi_waits(nc):
    """This image's walrus supports a single sync-wait slot per instruction.
    Hoist all-but-one wait of any multi-wait instruction onto single-wait
    NoOps on the same engine, placed immediately before it (same per-engine
    program order => equivalent synchronization)."""
    for f in nc.m.functions:
        for bb in f.blocks:
            insts = list(bb.instructions)
            if not any(ins.sync_info and len(ins.sync_info.on_wait) > 1
                       for ins in insts):
                continue
            new = []
            for ins in insts:
                si = ins.sync_info
                if si is not None and len(si.on_wait) > 1:
                    waits = list(si.on_wait)
                    for w in waits[:-1]:
                        new.append(mybir.InstNoOp(
                            name=nc.get_next_instruction_name(),
                            sync_info=mybir.SyncInfo(on_wait=[w], on_update=[]),
                            bass_nofuse=True,
                            engine=ins.engine,
                        ))
                    si.on_wait = [waits[-1]]
                new.append(ins)
            bb.instructions = new


def _ensure_ntff_hook():
    """The agent image's antenv lacks axon_hooks; bass_utils imports it when
    trace=True. Install a shim and, if possible, the real ctypes profiler."""
    import types
    try:
        import antenv.axon_hooks  # noqa: F401
        return
    except Exception:
        pass
    try:
        import antenv
        mod = types.ModuleType("antenv.axon_hooks")
        state = {"h": None}
        mod.set_axon_ntff_profile_hook = lambda h: state.__setitem__("h", h)
        mod.get_axon_ntff_profile_hook = lambda: state["h"]
        sys.modules["antenv.axon_hooks"] = mod
        antenv.axon_hooks = mod
        try:
            from trn_agent_boot.trn_boot import _ntff_profile_via_ctypes
            mod.set_axon_ntff_profile_hook(
                _ntff_profile_via_ctypes("/opt/axon/libaxon_pjrt.so"))
        except Exception:
            pass
    except Exception:
        pass


_ensure_ntff_hook()

F32 = mybir.dt.float32
BF16 = mybir.dt.bfloat16
RELU = mybir.ActivationFunctionType.Relu
ADD = mybir.AluOpType.add
SUB = mybir.AluOpType.subtract
MULT = mybir.AluOpType.mult
MAX = mybir.AluOpType.max

B = 64
NV = 256
N = 3 * NV  # 768
NCORES = 8
IPC = B // NCORES  # 8 items per core

LAST_RUN_INFO = {}

# Sparse corrections for one A application, coefficient-1 form, order-safe:
# (dst_col, 'S'|'U', src_col, op). S reads must precede writes to their col.
ENT_OPS = [
    (256, 'S', 100, ADD), (512, 'S', 200, ADD),
    (256, 'U', 255, SUB), (512, 'U', 511, SUB),
    (100, 'U', 256, ADD), (100, 'U', 257, ADD),
    (200, 'U', 512, ADD), (200, 'U', 513, ADD),
    (255, 'U', 256, SUB), (511, 'U', 512, SUB),
]


def _np_tri_shift(U):
    S = U.copy()
    S[..., 1:, :] += U[..., :-1, :]
    S[..., :-1, :] += U[..., 1:, :]
    return S


def _np_ents(S, U):
    for (j, kind, k, op) in ENT_OPS:
        src = (S if kind == 'S' else U)[..., k, :].copy()
        if op is ADD:
            S[..., j, :] += src
        else:
            S[..., j, :] -= src
    return S


def _structure_matches(A_norm, d):
    """Does d ⊙ (tri+ents)(d ⊙ Z) reproduce A_norm @ Z?"""
    rng = np.random.default_rng(12345)
    Z = rng.standard_normal((1, N, 4)).astype(np.float32)
    want = np.einsum('ij,bjf->bif', A_norm, Z)
    U = d[None, :, None] * Z
    got = d[None, :, None] * _np_ents(_np_tri_shift(U), U)
    scale = np.abs(want).max() + 1e-30
    return np.abs(want - got).max() / scale < 1e-4


# ---------------------------------------------------------------------------
# v3 fast-path program
# ---------------------------------------------------------------------------

def _build_program_v3(u2_dve=(3, 7), f1_dve_mod=4):
    nc = bass.Bass()

    xpk_d = nc.declare_dram_parameter("xpk", [2, 128, N], BF16, isOutput=False)
    w1rep_d = nc.declare_dram_parameter("w1rep", [128, 256], BF16, isOutput=False)
    w2p_d = nc.declare_dram_parameter("w2p", [128, 256], BF16, isOutput=False)
    wc_d = nc.declare_dram_parameter("wc", [128, 32], BF16, isOutput=False)
    d2w_d = nc.declare_dram_parameter("d2w", [128, 2 * N], BF16, isOutput=False)
    out_d = nc.declare_dram_parameter("outp", [128, 2 * N], BF16, isOutput=True)

    def ents(eng, Sv, Uv, i0, i1):
        # The 10 coefficient-1 corrections pair up into 5 two-column strided
        # ops (dst/src strides are independent). Order matches ENT_OPS.
        P = slice(0, 128)
        I = slice(i0, i1)
        pairs = [
            (Sv[P, I, 256:513:256], Sv[P, I, 100:201:100], ADD),
            (Sv[P, I, 256:513:256], Uv[P, I, 255:512:256], SUB),
            (Sv[P, I, 100:201:100], Uv[P, I, 256:513:256], ADD),
            (Sv[P, I, 100:201:100], Uv[P, I, 257:514:256], ADD),
            (Sv[P, I, 255:512:256], Uv[P, I, 256:513:256], SUB),
        ]
        for dst, s, op in pairs:
            eng.tensor_tensor(dst, dst, s, op=op)

    def tri(Sv, Uv, i0, i1):
        nc.vector.tensor_tensor(Sv[0:128, i0:i1, 1:N], Uv[0:128, i0:i1, 1:N],
                                Uv[0:128, i0:i1, 0:N - 1], op=ADD)
        nc.vector.tensor_copy(Sv[0:128, i0:i1, 0:1], Uv[0:128, i0:i1, 0:1])
        nc.vector.tensor_tensor(Sv[0:128, i0:i1, 0:N - 1],
                                Sv[0:128, i0:i1, 0:N - 1],
                                Uv[0:128, i0:i1, 1:N], op=ADD)

    with TileContext(nc) as tc:
        with (
            tc.tile_pool(name="const", bufs=1) as cpool,
            tc.tile_pool(name="psP", bufs=4, space="PSUM") as psP,
        ):
            xpk = cpool.tile([128, 2 * N], BF16)
            for g in range(2):
                nc.sync.dma_start(xpk[:, g * N:(g + 1) * N], xpk_d[g])
            w2p = cpool.tile([128, 256], BF16)
            nc.sync.dma_start(w2p[:, :], w2p_d[:, :])
            w1rep = cpool.tile([128, 256], BF16)
            nc.sync.dma_start(w1rep[:, :], w1rep_d[:, :])
            d2w = cpool.tile([128, 2 * N], BF16)
            nc.sync.dma_start(d2w[:, :], d2w_d[:, :])
            wc = cpool.tile([128, 32], BF16)
            nc.sync.dma_start(wc[:, :], wc_d[:, :])

            # PE warm-up burst: fills the idle PE window while DMAs + the L1
            # aggregation run, and ramps the PE toward its fast p-state.
            # One tile, WAW-chained matmuls, one tiny reader to keep the
            # pool-rotation dependency chain sound.
            wps = psP.tile([128, 1024], F32, tag="ps")
            for _ in range(10):
                nc.tensor.matmul(wps[:, 0:256], w2p[:, 0:128], w2p[:, :],
                                 start=True, stop=True)
            wsink = cpool.tile([128, 1], F32)
            nc.vector.tensor_copy(wsink[:, :], wps[:, 0:1])

            # ---- L1 aggregation (per group so feat1 g0 can start early) ---
            Xv = xpk[:, :].rearrange("p (g n) -> p g n", n=N)
            G = cpool.tile([128, 2 * N], BF16)
            Gv = G[:, :].rearrange("p (g n) -> p g n", n=N)
            Gp = cpool.tile([128, 2 * N], BF16)
            # group 0 column-chunked: all sparse-entry columns are < 514,
            # so tri+ents+scale of cols [0:512] complete early and feat1
            # g0's first (cs=0) matmuls start ~1.2us sooner.
            C1 = 514
            nc.vector.tensor_tensor(G[:, 1:C1], xpk[:, 1:C1],
                                    xpk[:, 0:C1 - 1], op=ADD)
            nc.vector.tensor_copy(G[:, 0:1], xpk[:, 0:1])
            nc.vector.tensor_tensor(G[:, 0:C1], G[:, 0:C1],
                                    xpk[:, 1:C1 + 1], op=ADD)
            ents(nc.vector, Gv, Xv, 0, 1)
            nc.vector.tensor_mul(Gp[:, 0:512], d2w[:, 0:512], G[:, 0:512])
            nc.vector.tensor_tensor(G[:, C1:N], xpk[:, C1:N],
                                    xpk[:, C1 - 1:N - 1], op=ADD)
            nc.vector.tensor_tensor(G[:, C1:N - 1], G[:, C1:N - 1],
                                    xpk[:, C1 + 1:N], op=ADD)
            nc.vector.tensor_mul(Gp[:, 512:N], d2w[:, 512:N], G[:, 512:N])
            # group 1 unchunked (off the critical head path)
            tri(Gv, Xv, 1, 2)
            ents(nc.vector, Gv, Xv, 1, 2)
            nc.vector.tensor_mul(Gp[:, N:2 * N], d2w[:, N:2 * N],
                                 G[:, N:2 * N])

            # ---- group-phase pipeline -------------------------------------
            # PE order: f1g0, f2g0, f1g1, f2g1, f4g0, f4g1 — so agg2 g0 (DVE)
            # overlaps feat1 g1 (PE/ACT), and agg2 g1 overlaps feat2 g1.
            # Long shifted adds are seam-split DVE/Pool (Pool takes the
            # high-column slice; sparse-entry columns all fall on the DVE
            # side, so ents never wait on Pool).
            h1 = [cpool.tile([128, IPC * N], BF16, name=f"h1_{h}")
                  for h in range(2)]
            u2 = cpool.tile([128, IPC * N], BF16)
            s2 = cpool.tile([128, IPC * N], BF16)
            U2v = u2[:, :].rearrange("p (i n) -> p i n", n=N)
            S2v = s2[:, :].rearrange("p (i n) -> p i n", n=N)
            u3 = cpool.tile([128, 2 * N], BF16)

            CUT = 624

            def tri_split(Sv, Uv, i0, i1):
                tri(Sv, Uv, i0, i1)

            def f1_group(g):
                for j in range(4):
                    it = g * 4 + j
                    for half in range(2):
                        ps = psP.tile([128, 1024], F32, tag="ps", name="psf1")
                        for cs, w in ((0, 512), (512, 256)):
                            nc.tensor.matmul(
                                ps[:, cs:cs + w],
                                w1rep[32 * j:32 * j + 6,
                                      half * 128:(half + 1) * 128],
                                Gp[32 * j:32 * j + 6,
                                   g * N + cs: g * N + cs + w],
                                start=True, stop=True,
                                tile_position=(32 * j, 0))
                        dst1 = h1[half][:, it * N:(it + 1) * N]
                        if (j * 2 + half) in (3, 7):
                            nc.vector.tensor_scalar(
                                dst1, ps[:, 0:N], 0.0, scalar2=None, op0=MAX)
                        else:
                            nc.scalar.activation(dst1, ps[:, 0:N], RELU)

            def f2_group(g):
                for j in range(4):
                    it = g * 4 + j
                    ps = psP.tile([128, 1024], F32, tag="ps", name="psf2")
                    for ns, w in ((0, 512), (1, 256)):
                        for kh in range(2):
                            nc.tensor.matmul(
                                ps[:, ns * 512:ns * 512 + w],
                                w2p[:, kh * 128:(kh + 1) * 128],
                                h1[kh][:, it * N + ns * 512:
                                       it * N + ns * 512 + w],
                                start=(kh == 0), stop=(kh == 1))
                    dst = u2[:, it * N:(it + 1) * N]
                    if j == 3 and g == 0:
                        nc.vector.tensor_copy(dst, ps[:, 0:N])
                    else:
                        nc.scalar.copy(dst, ps[:, 0:N])

            def agg2_group(g):
                tri_split(S2v, U2v, g * 4, g * 4 + 4)
                ents(nc.vector, S2v, U2v, g * 4, g * 4 + 4)
                sl = s2[:, g * 4 * N:(g + 1) * 4 * N]
                if g == 0:
                    nc.scalar.activation(sl, sl, RELU)
                else:
                    nc.vector.tensor_scalar(sl, sl, 0.0, scalar2=None, op0=MAX)

            def f4_group(g):
                ps = psP.tile([128, 1024], F32, tag="ps", name="psf4")
                for cs, w in ((0, 512), (512, 256)):
                    for j in range(4):
                        nc.tensor.matmul(
                            ps[32 * j:32 * j + 32, cs:cs + w],
                            wc[:, :],
                            s2[:, (g * 4 + j) * N + cs:
                               (g * 4 + j) * N + cs + w],
                            start=True, stop=True,
                            tile_position=(0, 32 * j))
                nc.vector.tensor_mul(u3[:, g * N:(g + 1) * N],
                                     d2w[:, 0:N], ps[:, 0:N])

            f1_group(0)
            f1_group(1)
            f2_group(0)
            agg2_group(0)
            f2_group(1)
            agg2_group(1)
            f4_group(0)
            f4_group(1)

            s3 = cpool.tile([128, 2 * N], BF16)
            U3v = u3[:, :].rearrange("p (g n) -> p g n", n=N)
            S3v = s3[:, :].rearrange("p (g n) -> p g n", n=N)
            tri_split(S3v, U3v, 0, 2)
            ents(nc.vector, S3v, U3v, 0, 2)
            m4 = cpool.tile([128, 2 * N], BF16)
            nc.vector.tensor_mul(m4[:, :], d2w[:, :], s3[:, :])
            M4v = m4[:, :].rearrange("p (g n) -> p g n", n=N)
            s4 = cpool.tile([128, 2 * N], BF16)
            S4v = s4[:, :].rearrange("p (g n) -> p g n", n=N)
            tri_split(S4v, M4v, 0, 2)
            ents(nc.vector, S4v, M4v, 0, 2)
            nc.sync.dma_start(out_d[:, :], s4[:, :])

    return nc


# ---------------------------------------------------------------------------
# v1 dense fallback (bias / unexpected adjacency)
# ---------------------------------------------------------------------------

def _build_program_v1(with_bias: bool):
    nc = bass.Bass()
    KT = N // 128

    xT_d = nc.declare_dram_parameter("xT", [IPC, 6, N], F32, isOutput=False)
    anT_d = nc.declare_dram_parameter("anT", [N, N], F32, isOutput=False)
    a2T_d = nc.declare_dram_parameter("a2T", [N, N], F32, isOutput=False)
    w1T_d = nc.declare_dram_parameter("w1T", [6, 256], F32, isOutput=False)
    w2Tp_d = nc.declare_dram_parameter("w2Tp", [128, 256], F32, isOutput=False)
    w34T_d = nc.declare_dram_parameter("w34T", [128, 3], F32, isOutput=False)
    if with_bias:
        p1t_d = nc.declare_dram_parameter("p1t", [128, 2 * N], F32, isOutput=False)
        p2t_d = nc.declare_dram_parameter("p2t", [128, N], F32, isOutput=False)
        cpt_d = nc.declare_dram_parameter("cpt", [3 * IPC, N], F32, isOutput=False)
    out_d = nc.declare_dram_parameter("outp", [3 * IPC, N], F32, isOutput=True)

    with TileContext(nc) as tc:
        with (
            tc.tile_pool(name="const", bufs=1) as cpool,
            tc.tile_pool(name="acts", bufs=2) as apool,
            tc.tile_pool(name="psf", bufs=2, space="PSUM") as psf,
            tc.tile_pool(name="psa", bufs=3, space="PSUM") as psa,
        ):
            anT = cpool.tile([128, KT * N], F32)
            nc.sync.dma_start(
                anT[:, :].rearrange("p (k j) -> p k j", j=N),
                anT_d[:, :].rearrange("(k p) j -> p k j", p=128))
            a2T = cpool.tile([128, KT * N], F32)
            nc.sync.dma_start(
                a2T[:, :].rearrange("p (k j) -> p k j", j=N),
                a2T_d[:, :].rearrange("(k p) j -> p k j", p=128))
            w1T = cpool.tile([6, 256], F32)
            nc.sync.dma_start(w1T[:, :], w1T_d[:, :])
            w2Tp = cpool.tile([128, 256], F32)
            nc.sync.dma_start(w2Tp[:, :], w2Tp_d[:, :])
            w34T = cpool.tile([128, 3], F32)
            nc.sync.dma_start(w34T[:, :], w34T_d[:, :])
            if with_bias:
                p1t = cpool.tile([128, 2 * N], F32)
                nc.sync.dma_start(p1t[:, :], p1t_d[:, :])
                p2t = cpool.tile([128, N], F32)
                nc.sync.dma_start(p2t[:, :], p2t_d[:, :])
                cpt = cpool.tile([3 * IPC, N], F32)
                nc.sync.dma_start(cpt[:, :], cpt_d[:, :])

            z34 = cpool.tile([128, KT * 3 * IPC], F32)

            for it in range(IPC):
                xT = apool.tile([6, N], F32, tag="xT")
                nc.sync.dma_start(xT[:, :], xT_d[it])

                z1 = apool.tile([128, KT * 256], F32, tag="z1")
                for m in range(KT):
                    ps = psf.tile([128, 256], F32, tag="feat")
                    nc.tensor.matmul(
                        ps[:, :], xT[:, m * 128:(m + 1) * 128], w1T[:, :],
                        start=True, stop=True,
                    )
                    nc.vector.tensor_copy(z1[:, m * 256:(m + 1) * 256], ps[:, :])

                h1t = apool.tile([128, 2 * N], F32, tag="h1t")
                for fh in range(2):
                    for ns in range(2):
                        ps = psa.tile([128, 384], F32, tag="agg")
                        for k in range(KT):
                            nc.tensor.matmul(
                                ps[:, :],
                                z1[:, k * 256 + fh * 128: k * 256 + fh * 128 + 128],
                                anT[:, k * N + ns * 384: k * N + ns * 384 + 384],
                                start=(k == 0), stop=(k == KT - 1),
                            )
                        dst = h1t[:, fh * N + ns * 384: fh * N + ns * 384 + 384]
                        if with_bias:
                            nc.vector.tensor_tensor(
                                dst, ps[:, :],
                                p1t[:, fh * N + ns * 384: fh * N + ns * 384 + 384],
                                op=ADD,
                            )
                            nc.scalar.activation(dst, dst, RELU)
                        else:
                            nc.scalar.activation(dst, ps[:, :], RELU)

                z2 = apool.tile([128, KT * 128], F32, tag="z2")
                for m in range(KT):
                    ps = psf.tile([128, 128], F32, tag="feat")
                    for kh in range(2):
                        nc.tensor.matmul(
                            ps[:, :],
                            h1t[:, kh * N + m * 128: kh * N + m * 128 + 128],
                            w2Tp[:, kh * 128:(kh + 1) * 128],
                            start=(kh == 0), stop=(kh == 1),
                        )
                    nc.vector.tensor_copy(z2[:, m * 128:(m + 1) * 128], ps[:, :])

                h2t = apool.tile([128, N], F32, tag="h2t")
                for ns in range(2):
                    ps = psa.tile([128, 384], F32, tag="agg")
                    for k in range(KT):
                        nc.tensor.matmul(
                            ps[:, :],
                            z2[:, k * 128:(k + 1) * 128],
                            anT[:, k * N + ns * 384: k * N + ns * 384 + 384],
                            start=(k == 0), stop=(k == KT - 1),
                        )
                    dst = h2t[:, ns * 384: ns * 384 + 384]
                    if with_bias:
                        nc.vector.tensor_tensor(
                            dst, ps[:, :], p2t[:, ns * 384: ns * 384 + 384],
                            op=ADD,
                        )
                        nc.scalar.activation(dst, dst, RELU)
                    else:
                        nc.scalar.activation(dst, ps[:, :], RELU)

                for m in range(KT):
                    ps = psf.tile([128, 3], F32, tag="feat")
                    nc.tensor.matmul(
                        ps[:, :], h2t[:, m * 128:(m + 1) * 128], w34T[:, :],
                        start=True, stop=True,
                    )
                    base = m * 3 * IPC + it * 3
                    nc.vector.tensor_copy(z34[:, base: base + 3], ps[:, :])

            outT = cpool.tile([3 * IPC, N], F32)
            for ns in range(2):
                ps = psa.tile([3 * IPC, 384], F32, tag="agg")
                for k in range(KT):
                    nc.tensor.matmul(
                        ps[:, :],
                        z34[:, k * 3 * IPC:(k + 1) * 3 * IPC],
                        a2T[:, k * N + ns * 384: k * N + ns * 384 + 384],
                        start=(k == 0), stop=(k == KT - 1),
                    )
                dst = outT[:, ns * 384: ns * 384 + 384]
                if with_bias:
                    nc.vector.tensor_tensor(
                        dst, ps[:, :], cpt[:, ns * 384: ns * 384 + 384],
                        op=ADD,
                    )
                else:
                    nc.vector.tensor_copy(dst, ps[:, :])
            nc.sync.dma_start(out_d[:, :], outT[:, :])

    return nc


def kernel(x, inputs, adjacency, W1, b1, W2, b2, W3, b3, W4, b4,
           parent_sel, child1_sel, child2_sel):
    global LAST_RUN_INFO
    x = np.asarray(x, np.float32)
    inp = np.asarray(inputs, np.float32)
    A = np.asarray(adjacency, np.float32)
    W1 = np.asarray(W1, np.float32); b1 = np.asarray(b1, np.float32)
    W2 = np.asarray(W2, np.float32); b2 = np.asarray(b2, np.float32)
    W3 = np.asarray(W3, np.float32); b3 = np.asarray(b3, np.float32)
    W4 = np.asarray(W4, np.float32); b4 = np.asarray(b4, np.float32)
    parent_sel = np.asarray(parent_sel, np.int64)
    child1_sel = np.asarray(child1_sel, np.int64)
    child2_sel = np.asarray(child2_sel, np.int64)

    clamp_rows = np.concatenate([
        parent_sel, NV + child1_sel, 2 * NV + child2_sel,
    ]).astype(np.int64)

    x0 = x.copy()
    x0[:, clamp_rows, 0:3] = inp[:, clamp_rows, :]

    deg = A.sum(axis=-1)
    deg_safe = np.where(deg == 0, np.float32(1.0), deg)
    d = np.where(deg == 0, np.float32(0.0),
                 deg_safe ** np.float32(-0.5)).astype(np.float32)
    A_norm = (A * d[:, None] * d[None, :]).astype(np.float32)

    with_bias = bool(np.any(b1) or np.any(b2) or np.any(b3) or np.any(b4))
    use_v3 = (not with_bias) and _structure_matches(A_norm, d)

    trace = os.environ.get("KERNEL_TRACE", "") == "1"

    if use_v3:
        bf = ml_dtypes.bfloat16
        # Xd = d (.) x0, packed: [core, g, 32j+f, n]
        Xd = (d[None, :, None] * x0).astype(bf)                  # (B, N, 6)
        xpk = np.zeros((NCORES, 2, 128, N), bf)
        Xr = Xd.reshape(NCORES, 2, 4, N, 6).transpose(0, 1, 2, 4, 3)
        xpk.reshape(NCORES, 2, 4, 32, N)[:, :, :, 0:6, :] = Xr

        w1rep = np.zeros((128, 256), bf)
        w1rep.reshape(4, 32, 256)[:, 0:6, :] = W1.T.astype(bf)[None]
        w2p = np.ascontiguousarray(
            W2.T.reshape(2, 128, 128).transpose(1, 0, 2).reshape(128, 256)
        ).astype(bf)
        wc = np.zeros((128, 32), bf)                             # (128, 32)
        wc[:, 0:3] = (W3.T @ W4.T).astype(bf)
        d2w = np.ascontiguousarray(
            np.broadcast_to(np.tile((d * d).astype(np.float32), 2),
                            (128, 2 * N))).astype(bf)

        nc = _build_program_v3()
        _split_multi_waits(nc)
        in_maps = [{
            "xpk": xpk[c], "w1rep": w1rep, "w2p": w2p, "wc": wc, "d2w": d2w,
        } for c in range(NCORES)]

        res = run_bass_kernel_spmd(nc, in_maps, list(range(NCORES)),
                                   trace=trace)
        LAST_RUN_INFO = {
            "exec_time_ns": res.exec_time_ns,
            "mean_exec_time_ns": res.mean_exec_time_ns,
            "max_exec_time_core_id": res.max_exec_time_core_id,
        }

        out = np.empty((B, N, 3), np.float32)
        for c in range(NCORES):
            o = np.asarray(res.results[c]["outp"], bf).astype(np.float32)
            # o[32j+s, g*768+n] -> out[item, n, s]
            oi = o.reshape(4, 32, 2, N)[:, 0:3, :, :]       # (j, s, g, n)
            arr = oi.transpose(2, 0, 3, 1).reshape(IPC, N, 3)
            out[c * IPC:(c + 1) * IPC] = arr
        out *= d[None, :, None]
    else:
        AnT = np.ascontiguousarray(A_norm.T)
        A2T = np.ascontiguousarray((A_norm @ A_norm).T.astype(np.float32))
        W1T = np.ascontiguousarray(W1.T)
        W2Tp = np.ascontiguousarray(
            W2.T.reshape(2, 128, 128).transpose(1, 0, 2).reshape(128, 256))
        W34T = np.ascontiguousarray(W3.T @ W4.T)

        extra = {}
        if with_bias:
            s = A_norm.sum(axis=1).astype(np.float32)
            s2 = (A_norm @ s).astype(np.float32)
            p1t = np.einsum('f,n->fn', b1, s).astype(np.float32)
            p1t = p1t.reshape(2, 128, N).transpose(1, 0, 2).reshape(128, 2 * N)
            p2t = np.einsum('f,n->fn', b2, s).astype(np.float32)
            cp = (np.einsum('f,n->fn', W4 @ b3, s2) +
                  np.einsum('f,n->fn', b4, s)).astype(np.float32)
            cpt = np.tile(cp, (IPC, 1)).astype(np.float32)
            extra = {"p1t": np.ascontiguousarray(p1t),
                     "p2t": np.ascontiguousarray(p2t),
                     "cpt": np.ascontiguousarray(cpt)}

        xT_all = np.ascontiguousarray(
            x0.transpose(0, 2, 1).reshape(NCORES, IPC, 6, N))

        nc = _build_program_v1(with_bias)
        _split_multi_waits(nc)

        in_maps = []
        for c in range(NCORES):
            m = {
                "xT": xT_all[c], "anT": AnT, "a2T": A2T,
                "w1T": W1T, "w2Tp": W2Tp, "w34T": W34T,
            }
            m.update(extra)
            in_maps.append(m)

        res = run_bass_kernel_spmd(nc, in_maps, list(range(NCORES)),
                                   trace=trace)
        LAST_RUN_INFO = {
            "exec_time_ns": res.exec_time_ns,
            "mean_exec_time_ns": res.mean_exec_time_ns,
            "max_exec_time_core_id": res.max_exec_time_core_id,
        }

        out = np.empty((B, N, 3), np.float32)
        for c in range(NCORES):
            o = res.results[c]["outp"]
            for it in range(IPC):
                out[c * IPC + it] = o[it * 3:(it + 1) * 3, :].T

    out[:, clamp_rows, :] = inp[:, clamp_rows, :]
    return out
